# revision 14
# baseline (speedup 1.0000x reference)
"""DyGATFR (focal-reweighted dynamic GAT) Trainium2 kernel.

kernel(**inputs) takes the FULL inputs (x (50000,256) f32, edge_index
(2,800000) int64, params pytree) and returns the FULL output (50000,1)
f32.  Internally: nodes are block-partitioned across 8 NeuronCores (6250
per core, padded to 6272 = 49 blocks of 128), edges are sharded by dst
owner, sorted by dst block, and padded to a uniform T tiles of 128 edges
per block; one SPMD Bass program runs on cores 0-7.

Per conv layer, each core:
  1. computes xh^T = lin^T @ h^T feature-major on the tensor engine,
  2. assembles a node-major [xh | a_src] table slab via PE transposes,
  3. AllGathers slabs into a per-core replicated global table in HBM,
  4. indirect-DMA-gathers one table row per in-edge and 20B of dst-side
     values ([a_dst, focal-scale]) per edge,
  5. forms logits e = leaky(a_src+a_dst)*(1+focal(dst)), ex = exp(e)
     (no segment-max: logits are bounded by ~1 in magnitude, softmax is
     shift-invariant, and the reference's 1e-16 denominator eps is
     negligible against denom >= exp(min logit)),
  6. scatter-adds [ex*xh | ex] into a per-dst-block PSUM accumulator via
     one-hot selection-matrix matmuls (128 edges x 128 dst nodes),
  7. divides by the per-(node,head) denominator, then LN/gelu/residual.
Feature-major row-broadcasts (LN stats, softmax denominators) are
realized with all-ones matmuls since DVE lanes cannot read across
partitions.
"""

import os
import ml_dtypes
import numpy as np

from concourse import bass, bacc, mybir, tile
from concourse.bass_utils import run_bass_kernel_spmd
from concourse.masks import make_identity

dt = mybir.dt
Alu = mybir.AluOpType
Act = mybir.ActivationFunctionType

IN, HID, OUT, HEADS = 256, 128, 64, 4
HEAD_DIMS = (32, 32, 16)
FOUTS = (128, 128, 64)
NPROTO = 8
F_ALPHA, LN_EPS = 0.25, 1e-5
NCORES = 8
DENOM_EPS = 1e-16


class Cfg:
    def __init__(self, n, e, t_override=None):
        self.N = n
        self.E = e
        assert n % NCORES == 0
        self.npc = n // NCORES                     # true nodes per core
        self.nl = ((self.npc + 127) // 128) * 128  # padded local rows
        self.nb = self.nl // 128                   # dst blocks per core
        self.ng = NCORES * self.nl                 # global table rows
        self.t_override = t_override


FULL = Cfg(50000, 800000)


# --------------------------------------------------------------------------
# host-side preprocessing
# --------------------------------------------------------------------------

def wrap16(vals):
    """int16 index list -> dma_gather idx layout (128, n//16):
    idx i lives at [16k + i%16, i//16] for every k (replicated per Q7 core)."""
    n = vals.shape[0]
    assert n % 16 == 0
    w = vals.reshape(n // 16, 16).T.astype(np.int16)
    return np.tile(w, (8, 1))


def gid_of(node, cfg):
    """Global table row (AllGather is chunked in 4 row-quarters)."""
    c = node // cfg.npc
    dloc = node - c * cfg.npc
    q = dloc // (cfg.nl // 4)
    r = dloc % (cfg.nl // 4)
    return (q * NCORES + c) * (cfg.nl // 4) + r


def prep_edges(edge_index, cfg):
    """Shard edges by dst owner, group by dst block, split by src gid half
    (int16 gather indices), pad to uniform T_lo/T_hi tiles per block."""
    src = np.asarray(edge_index[0], dtype=np.int64)
    dst = np.asarray(edge_index[1], dtype=np.int64)
    npc, nb = cfg.npc, cfg.nb
    half = cfg.ng // 2

    owner = dst // npc
    sgid_all = gid_of(src, cfg)

    per_core = []
    tlo = thi = 1
    for c in range(NCORES):
        sel = np.nonzero(owner == c)[0]
        dloc = dst[sel] - c * npc
        gid = sgid_all[sel]
        lo = gid < half
        blk = dloc >> 7
        # order: block asc, then lo-before-hi
        order = np.lexsort((~lo, blk))
        sel, dloc, gid, lo, blk = (a[order] for a in (sel, dloc, gid, lo, blk))
        nlo = np.bincount(blk[lo], minlength=nb)
        nhi = np.bincount(blk[~lo], minlength=nb)
        tlo = max(tlo, int(np.ceil(nlo.max() / 128)))
        thi = max(thi, int(np.ceil(nhi.max() / 128)))
        per_core.append((dloc, gid, lo, blk, nlo, nhi))

    T_lo, T_hi = tlo, thi
    T = T_lo + T_hi
    cfg.T_lo, cfg.T_hi = T_lo, T_hi

    out = []
    for c in range(NCORES):
        dloc, gid, lo, blk, nlo, nhi = per_core[c]
        caplo, caphi = T_lo * 128, T_hi * 128
        sgl = np.zeros((nb, caplo), np.int16)
        sgh = np.zeros((nb, caphi), np.int16)
        dr = np.full((nb, T * 128), 300.0, np.float32)
        si = np.zeros((nb, T * 128), np.int16)
        starts = np.concatenate([[0], np.cumsum(nlo + nhi)])
        for b in range(nb):
            s = starts[b]
            a, m = int(nlo[b]), int(nhi[b])
            sgl[b, :a] = gid[s:s + a]
            sgh[b, :m] = gid[s + a:s + a + m] - half
            dr[b, :a] = (dloc[s:s + a] & 127).astype(np.float32)
            dr[b, caplo:caplo + m] = (dloc[s + a:s + a + m] & 127)
            si[b, :a] = dloc[s:s + a]
            si[b, caplo:caplo + m] = dloc[s + a:s + a + m]

        def dev_pt(a2, t):
            # slot i -> (tile i//128, partition i%128); device [p, b*t + i//128]
            return np.ascontiguousarray(
                a2.reshape(nb, t, 128).transpose(2, 0, 1).reshape(128, nb * t))

        out.append({
            "sgl": np.hstack([wrap16(sgl[b]) for b in range(nb)]),
            "sgh": np.hstack([wrap16(sgh[b]) for b in range(nb)]),
            "sdl": np.hstack([wrap16(si[b]) for b in range(nb)]),
            "dr": dev_pt(dr, T).astype(ml_dtypes.bfloat16),
        })
    return T, out


def prep_weights(params):
    w = {}
    p = params

    def a(name, arr):
        w[name] = np.ascontiguousarray(np.asarray(arr, dtype=np.float32))

    pack = lambda arr: np.asarray(arr, np.float32).reshape(
        2, 128, -1).transpose(1, 0, 2).reshape(128, -1)
    a("pe_w1", pack(p["pe_w1"]))
    a("pe_b1", np.reshape(p["pe_b1"], (64, 1)))
    a("pe_w2", p["pe_w2"])
    a("pe_b2", np.reshape(p["pe_b2"], (1, 1)))
    a("in_w", pack(p["in_w"]))
    a("in_b", np.reshape(p["in_b"], (HID, 1)))
    a("in_g", np.reshape(p["in_g"], (HID, 1)))
    a("in_beta", np.reshape(p["in_beta"], (HID, 1)))
    for l, cp in enumerate(p["convs"]):
        hd = HEAD_DIMS[l]
        f = HEADS * hd
        a(f"lin{l}", cp["lin"])
        asrc = np.zeros((f, HEADS), np.float32)
        adst = np.zeros((f, HEADS), np.float32)
        att_s = np.asarray(cp["att_src"], np.float32)
        att_d = np.asarray(cp["att_dst"], np.float32)
        for h in range(HEADS):
            asrc[h * hd:(h + 1) * hd, h] = att_s[h]
            adst[h * hd:(h + 1) * hd, h] = att_d[h]
        a(f"asrc{l}", asrc)
        a(f"adst{l}", adst)
        a(f"ng{l}", np.reshape(cp["ng"], (f, 1)))
        a(f"nbeta{l}", np.reshape(cp["nb"], (f, 1)))
    a("res_w", p["res_w"])
    a("res_b", np.reshape(p["res_b"], (OUT, 1)))
    a("q_w", p["q_w"])
    a("q_b", np.reshape(p["q_b"], (OUT, 1)))
    a("proto_kT", np.asarray(p["proto_k"]).T)
    a("proto_v", p["proto_v"])
    a("gate_w1", np.asarray(p["gate_w1"])[0:OUT, :])
    a("gate_w1p", np.asarray(p["gate_w1"])[OUT:OUT + 1, :])
    a("gate_b1", np.reshape(p["gate_b1"], (16, 1)))
    a("gate_w2", p["gate_w2"])
    a("gate_b2", np.reshape(p["gate_b2"], (64, 1)))
    a("cls_w1", p["cls_w1"])
    a("cls_b1", np.reshape(p["cls_b1"], (32, 1)))
    a("cls_w2", p["cls_w2"])
    a("cls_b2", np.reshape(p["cls_b2"], (1, 1)))
    w["iota128"] = np.broadcast_to(
        np.arange(128, dtype=np.float32), (128, 128)).astype(
        ml_dtypes.bfloat16).copy()
    return w


WEIGHT_SHAPES = {
    "pe_w1": (128, 128), "pe_b1": (64, 1), "pe_w2": (64, 1), "pe_b2": (1, 1),
    "in_w": (128, 2 * HID), "in_b": (HID, 1), "in_g": (HID, 1),
    "in_beta": (HID, 1),
    "res_w": (HID, OUT), "res_b": (OUT, 1),
    "q_w": (OUT, OUT), "q_b": (OUT, 1),
    "proto_kT": (OUT, NPROTO), "proto_v": (NPROTO, OUT),
    "gate_w1": (OUT, 16), "gate_w1p": (1, 16),
    "gate_b1": (16, 1),
    "gate_w2": (16, OUT), "gate_b2": (OUT, 1),
    "cls_w1": (OUT, 32), "cls_b1": (32, 1),
    "cls_w2": (32, 1), "cls_b2": (1, 1),
    "iota128": (128, 128),
}
for _l in range(3):
    _f = FOUTS[_l]
    WEIGHT_SHAPES.update({
        f"lin{_l}": (HID, _f), f"asrc{_l}": (_f, HEADS),
        f"adst{_l}": (_f, HEADS), f"ng{_l}": (_f, 1), f"nbeta{_l}": (_f, 1)})


def prep_inputs(x, edge_index, params, cfg):
    T, edata = prep_edges(edge_index, cfg)
    w = prep_weights(params)
    x = np.asarray(x, dtype=np.float32)
    in_maps = []
    for c in range(NCORES):
        m = dict(w)
        xt = np.zeros((2, 128, cfg.nl), np.float32)
        xt[:, :, :cfg.npc] = x[c * cfg.npc:(c + 1) * cfg.npc].T.reshape(
            2, 128, cfg.npc)
        xt = np.ascontiguousarray(xt.transpose(1, 0, 2).reshape(
            128, 2 * cfg.nl))
        m["xt"] = xt
        m.update(edata[c])
        in_maps.append(m)
    return T, in_maps


# --------------------------------------------------------------------------
# device program
# --------------------------------------------------------------------------

def _col_chunks(nl, width=512):
    out, c0 = [], 0
    while c0 < nl:
        out.append((c0, min(width, nl - c0)))
        c0 += width
    return out


def build_program(cfg, T):
    import contextlib
    from concourse.library_config import mlp

    nl, nb, ng = cfg.nl, cfg.nb, cfg.ng
    T_lo, T_hi = cfg.T_lo, cfg.T_hi
    half = ng // 2
    nq = nl // 4                       # AllGather row-chunk size (per rank)
    f32, bf16, i16 = dt.float32, dt.bfloat16, dt.int16
    nc = bacc.Bacc("TRN2", target_bir_lowering=False, debug=False,
                   num_devices=NCORES)

    shapes = dict(WEIGHT_SHAPES)
    shapes.update({"xt": (128, 2 * nl),
                   "sgl": (128, nb * T_lo * 8), "sgh": (128, nb * T_hi * 8),
                   "sdl": (128, nb * T * 8), "dr": (128, nb * T)})
    dtypes = {n: f32 for n in shapes}
    dtypes.update({"sgl": i16, "sgh": i16, "sdl": i16, "dr": bf16,
                   "iota128": bf16})
    ext = {}
    for name, shp in shapes.items():
        ext[name] = nc.dram_tensor(name, list(shp), dtypes[name],
                                   kind="ExternalInput").ap()
    out_ext = nc.dram_tensor("out", [cfg.npc, 1], f32,
                             kind="ExternalOutput").ap()

    # table rows (bf16): [xh(fo) | a_src(4) | a_dst(4) | fs | junk]
    EWS = [256, 256, 128]
    slab, table = [], []
    for l in range(3):
        slab.append(nc.dram_tensor(f"slab{l}", [nl, EWS[l]], bf16))
        table.append(nc.dram_tensor(f"table{l}", [ng, EWS[l]], bf16,
                                    addr_space="Shared"))

    chunks = _col_chunks(nl)

    with tile.TileContext(nc) as tc, contextlib.ExitStack() as st:
        cpool = st.enter_context(tc.tile_pool(name="consts", bufs=1))
        big = st.enter_context(tc.tile_pool(name="big", bufs=1))
        ppost = st.enter_context(tc.tile_pool(name="ppost", bufs=2,
                                              space="PSUM"))
        pagg = st.enter_context(tc.tile_pool(name="pagg", bufs=2,
                                             space="PSUM"))
        pdense = st.enter_context(tc.tile_pool(name="pdense", bufs=4,
                                               space="PSUM"))

        def ps(parts, width=512, name="ps"):
            return pdense.tile([parts, width], f32, name=name, tag="ps")

        nc.gpsimd.load_library(mlp)
        W = {}
        for n in shapes:
            if n in ("xt", "sgl", "sgh", "sdl", "dr"):
                continue
            W[n] = cpool.tile(list(shapes[n]), dtypes[n], name=n)
            nc.sync.dma_start(out=W[n][:], in_=ext[n][:])
        idn = cpool.tile([128, 128], f32, name="idn")
        make_identity(nc, idn[:])
        onesM = cpool.tile([128, 128], f32, name="onesM")
        nc.vector.memset(onesM[:], 1.0)
        zcol = cpool.tile([128, 1], f32, name="zcol")
        nc.vector.memset(zcol[:], 0.0)
        ecol = cpool.tile([128, 1], f32, name="ecol")
        nc.vector.memset(ecol[:], LN_EPS)
        nc.const_aps.aps[(f32, 0.0)] = zcol[:]
        nc.const_aps.aps[(f32, LN_EPS)] = ecol[:]

        drl = cpool.tile([128, nb * T], bf16, name="drl")
        nc.sync.dma_start(out=drl[:], in_=ext["dr"][:])

        # persistent activations
        hA = big.tile([HID, nl], f32, name="hA")
        hB = big.tile([HID, nl], f32, name="hB")
        h3T = big.tile([OUT, nl], f32, name="h3T")
        pT = big.tile([1, nl], f32, name="pT")
        p_nm = big.tile([128, nb], f32, name="p_nm")
        fs_nm = big.tile([128, nb], f32, name="fs_nm")

        def ts(out, in0, s1, op0, s2=None, op1=None):
            if s2 is None:
                nc.vector.tensor_scalar(out=out, in0=in0, scalar1=s1,
                                        scalar2=None, op0=op0)
            else:
                nc.vector.tensor_scalar(out=out, in0=in0, scalar1=s1,
                                        scalar2=s2, op0=op0, op1=op1)

        def tt(out, in0, in1, op):
            nc.vector.tensor_tensor(out=out, in0=in0, in1=in1, op=op)

        # ---- stage 1+2 fused: p head, input projection + LN + gelu -> hA
        with tc.tile_pool(name="s12", bufs=3) as wk:
            for c0, cw in chunks:
                xc = wk.tile([128, 1024], f32, name="xc", tag="xc")
                nc.sync.dma_start(out=xc[:, 0:cw], in_=ext["xt"][:, c0:c0 + cw])
                nc.sync.dma_start(out=xc[:, 512:512 + cw],
                                  in_=ext["xt"][:, nl + c0:nl + c0 + cw])
                p1 = ps(64, name="p_pe1")
                nc.tensor.matmul(p1[0:64, :cw], W["pe_w1"][:, 0:64],
                                 xc[:, 0:cw], start=True, stop=False)
                nc.tensor.matmul(p1[0:64, :cw], W["pe_w1"][:, 64:128],
                                 xc[:, 512:512 + cw], start=False, stop=True)
                t1c = wk.tile([64, 512], f32, name="t1c", tag="t1c")
                nc.scalar.activation(t1c[:, :cw], p1[0:64, :cw], Act.Relu,
                                     bias=W["pe_b1"][:, 0:1], scale=1.0)
                p2 = ps(1, name="p_pe2")
                nc.tensor.matmul(p2[0:1, :cw], W["pe_w2"][:], t1c[:, :cw],
                                 start=True, stop=True)
                nc.scalar.activation(pT[:, c0:c0 + cw], p2[0:1, :cw],
                                     Act.Sigmoid, bias=W["pe_b2"][0:1, 0:1],
                                     scale=1.0)
                pin = ps(HID, name="p_in")
                nc.tensor.matmul(pin[:, :cw], W["in_w"][:, 0:HID],
                                 xc[:, 0:cw], start=True, stop=False)
                nc.tensor.matmul(pin[:, :cw], W["in_w"][:, HID:2 * HID],
                                 xc[:, 512:512 + cw], start=False, stop=True)
                hc = wk.tile([HID, 512], f32, name="hc", tag="hc")
                nc.scalar.activation(hc[:, :cw], pin[:, :cw], Act.Identity,
                                     bias=W["in_b"][:, 0:1], scale=1.0)
                sqc = wk.tile([HID, 512], f32, name="sqc", tag="sqc")
                tt(sqc[:, :cw], hc[:, :cw], hc[:, :cw], Alu.mult)
                ps1 = ps(128, name="p_s1")
                ps2 = ps(128, name="p_s2")
                nc.tensor.matmul(ps1[:, :cw], onesM[0:HID, :], hc[:, :cw],
                                 start=True, stop=True)
                nc.tensor.matmul(ps2[:, :cw], onesM[0:HID, :], sqc[:, :cw],
                                 start=True, stop=True)
                mc = wk.tile([HID, 512], f32, name="mc", tag="mc")
                sc = wk.tile([HID, 512], f32, name="sc", tag="sc")
                nc.scalar.activation(mc[:, :cw], ps1[:, :cw], Act.Identity,
                                     bias=0.0, scale=1.0 / HID)
                nc.scalar.activation(sc[:, :cw], ps2[:, :cw], Act.Identity,
                                     bias=0.0, scale=1.0 / HID)
                tt(hc[:, :cw], hc[:, :cw], mc[:, :cw], Alu.subtract)
                tt(mc[:, :cw], mc[:, :cw], mc[:, :cw], Alu.mult)
                tt(sc[:, :cw], sc[:, :cw], mc[:, :cw], Alu.subtract)
                nc.scalar.activation(sc[:, :cw], sc[:, :cw], Act.Sqrt,
                                     bias=LN_EPS, scale=1.0)
                nc.vector.reciprocal(out=sc[:, :cw], in_=sc[:, :cw])
                tt(hc[:, :cw], hc[:, :cw], sc[:, :cw], Alu.mult)
                ts(hc[:, :cw], hc[:, :cw], W["in_g"][:, 0:1], Alu.mult,
                   W["in_beta"][:, 0:1], Alu.add)
                nc.scalar.activation(hA[:, c0:c0 + cw], hc[:, :cw], Act.Gelu,
                                     bias=0.0, scale=1.0)
            for b in range(nb):
                ppb = ppost.tile([128, 1], f32, name="ppb", tag="pt")
                nc.tensor.transpose(ppb[:], pT[0:1, b * 128:(b + 1) * 128],
                                    idn[0:1, 0:1])
                nc.scalar.activation(p_nm[:, b:b + 1], ppb[:], Act.Identity,
                                     bias=0.0, scale=1.0)
            ts(fs_nm[:], p_nm[:], -1.0, Alu.mult, 1.0, Alu.add)
            tt(fs_nm[:], fs_nm[:], fs_nm[:], Alu.mult)
            ts(fs_nm[:], fs_nm[:], F_ALPHA, Alu.mult, 1.0, Alu.add)

        # ---- conv layers
        cur, other = hA, hB
        for l in range(3):
            fo = FOUTS[l]
            hd = HEAD_DIMS[l]
            tw = fo + HEADS
            ew = EWS[l]
            _lyr_cm = tc.tile_pool(name=f"lyr{l}", bufs=3)
            lyr = _lyr_cm.__enter__()
            slab_v = slab[l][:]

            # dense: xh, a_src, a_dst; assemble slab rows per block
            for c0, cw in chunks:
                pxh = ps(fo, name="p_xh")
                nc.tensor.matmul(pxh[0:fo, :cw], W[f"lin{l}"][:],
                                 cur[:, c0:c0 + cw], start=True, stop=True)
                xhc = lyr.tile([fo, 512], f32, name="xhc", tag="xhc")
                nc.scalar.activation(xhc[:, :cw], pxh[0:fo, :cw],
                                     Act.Identity, bias=0.0, scale=1.0)
                pas = ps(HEADS, name="p_as")
                nc.tensor.matmul(pas[0:HEADS, :cw], W[f"asrc{l}"][:],
                                 xhc[:, :cw], start=True, stop=True)
                asc = lyr.tile([HEADS, 512], f32, name="asc", tag="asc")
                nc.scalar.activation(asc[:, :cw], pas[0:HEADS, :cw],
                                     Act.Identity, bias=0.0, scale=1.0)
                pad_ = ps(HEADS, name="p_ad")
                nc.tensor.matmul(pad_[0:HEADS, :cw], W[f"adst{l}"][:],
                                 xhc[:, :cw], start=True, stop=True)
                adc = lyr.tile([HEADS, 512], f32, name="adc", tag="adc")
                nc.scalar.activation(adc[:, :cw], pad_[0:HEADS, :cw],
                                     Act.Identity, bias=0.0, scale=1.0)
                for j in range(cw // 128):
                    b = c0 // 128 + j
                    jj = j * 128
                    pt = ppost.tile([128, tw + 4], f32, name="pt", tag="pt")
                    nc.tensor.transpose(pt[:, 0:fo],
                                        xhc[:, jj:jj + 128], idn[0:fo, 0:fo])
                    nc.tensor.transpose(pt[:, fo:fo + 4],
                                        asc[:, jj:jj + 128],
                                        idn[0:HEADS, 0:HEADS])
                    nc.tensor.transpose(pt[:, fo + 4:fo + 8],
                                        adc[:, jj:jj + 128],
                                        idn[0:HEADS, 0:HEADS])
                    tabc = lyr.tile([128, ew], bf16, name="tabc",
                                    tag="tabc")
                    nc.scalar.activation(tabc[:, 0:fo + 8], pt[:],
                                         Act.Identity, bias=0.0, scale=1.0)
                    nc.vector.tensor_copy(out=tabc[:, fo + 8:fo + 9],
                                          in_=fs_nm[:, b:b + 1])
                    nc.vector.memset(tabc[:, fo + 9:ew], 0.0)
                    nc.sync.dma_start(
                        out=slab_v[b * 128:(b + 1) * 128, :],
                        in_=tabc[:])
            for q in range(4):
                nc.gpsimd.collective_compute(
                    "AllGather", Alu.bypass,
                    replica_groups=[list(range(NCORES))],
                    ins=[slab_v[q * nq:(q + 1) * nq, :]],
                    outs=[table[l][q * NCORES * nq:(q + 1) * NCORES * nq, :]])

            # edge phase
            agg = lyr.tile([128, nb * 128], f32, name="agg", tag="agg",
                           bufs=1)
            aggv = agg[:].rearrange("p (b f) -> p b f", f=128)
            sd_off = 4 if l < 2 else fo + 4
            for b in range(nb):
                ixl = lyr.tile([128, T_lo * 8], i16, name="ixl", tag="ixl")
                nc.sync.dma_start(
                    out=ixl[:],
                    in_=ext["sgl"][:, b * T_lo * 8:(b + 1) * T_lo * 8])
                ixh = lyr.tile([128, T_hi * 8], i16, name="ixh", tag="ixh")
                nc.sync.dma_start(
                    out=ixh[:],
                    in_=ext["sgh"][:, b * T_hi * 8:(b + 1) * T_hi * 8])
                ixs = lyr.tile([128, T * 8], i16, name="ixs", tag="ixs")
                nc.sync.dma_start(
                    out=ixs[:], in_=ext["sdl"][:, b * T * 8:(b + 1) * T * 8])
                gb = lyr.tile([128, T * ew], bf16, name="gb", tag="gb",
                              bufs=2)
                nc.gpsimd.dma_gather(
                    gb[:, 0:T_lo * ew].rearrange("p (t w) -> p t w", w=ew),
                    table[l][0:half, :], ixl[:], T_lo * 128, T_lo * 128, ew,
                    single_packet=False)
                nc.gpsimd.dma_gather(
                    gb[:, T_lo * ew:T * ew].rearrange(
                        "p (t w) -> p t w", w=ew),
                    table[l][half:ng, :], ixh[:], T_hi * 128, T_hi * 128, ew,
                    single_packet=False)
                sw = 128
                sd = lyr.tile([128, T * sw], bf16, name="sd", tag="sd",
                              bufs=2)
                if l < 2:
                    side_src = slab_v[:, 128:256]
                    side_step = 256
                else:
                    side_src = slab_v
                    side_step = 128
                nc.gpsimd.dma_gather(
                    sd[:].rearrange("p (t w) -> p t w", w=sw),
                    side_src, ixs[:], T * 128, T * 128, sw,
                    elem_step=side_step, single_packet=False)
                gbv = gb[:].rearrange("p (t w) -> p t w", w=ew)
                sdv = sd[:].rearrange("p (t w) -> p t w", w=sw)

                ls = lyr.tile([128, T * HEADS], f32, name="ls", tag="ls")
                lsv = ls[:].rearrange("p (t h) -> p t h", h=HEADS)
                tt(lsv, gbv[:, :, fo:fo + 4], sdv[:, :, sd_off:sd_off + 4],
                   Alu.add)
                lk = lyr.tile([128, T * HEADS], f32, name="lk", tag="lk")
                ts(lk[:], ls[:], 0.2, Alu.mult)
                tt(ls[:], ls[:], lk[:], Alu.max)
                tt(lsv, lsv,
                   sdv[:, :, sd_off + 4:sd_off + 5].to_broadcast(
                       [128, T, HEADS]), Alu.mult)
                exf = lyr.tile([128, T * HEADS], bf16, name="exf", tag="exf")
                nc.scalar.activation(exf[:], ls[:], Act.Exp, bias=0.0,
                                     scale=1.0)

                pacc = pagg.tile([128, tw], f32, name="pacc", tag="pacc")
                for t in range(T):
                    sel = lyr.tile([128, 128], bf16, name="sel", tag="sel",
                                   bufs=4)
                    tt(sel[:], W["iota128"][:],
                       drl[:, b * T + t:b * T + t + 1].to_broadcast(
                           [128, 128]), Alu.is_equal)
                    msg = lyr.tile([128, tw], bf16, name="msg", tag="msg",
                                   bufs=4)
                    tt(msg[:, 0:fo].rearrange("p (h d) -> p h d", h=HEADS),
                       gb[:, t * ew:t * ew + fo].rearrange(
                           "p (h d) -> p h d", h=HEADS),
                       exf[:, t * HEADS:(t + 1) * HEADS].to_broadcast(
                           [128, HEADS, hd]),
                       Alu.mult)
                    nc.vector.tensor_copy(
                        out=msg[:, fo:tw],
                        in_=exf[:, t * HEADS:(t + 1) * HEADS])
                    nc.tensor.matmul(pacc[:], sel[:], msg[:],
                                     start=(t == 0), stop=(t == T - 1))
                rec = lyr.tile([128, HEADS], f32, name="rec", tag="rec")
                ts(rec[:], pacc[:, fo:tw], DENOM_EPS, Alu.add)
                nc.vector.reciprocal(out=rec[:], in_=rec[:])
                tt(aggv[:, b:b + 1, 0:fo].rearrange(
                    "p one (h d) -> p (one h) d", h=HEADS),
                   pacc[:, 0:fo].rearrange("p (h d) -> p h d", h=HEADS),
                   rec[:].to_broadcast([128, HEADS, hd]),
                   Alu.mult)

            # post: LN (node-major) -> transpose -> gelu -> residual
            mean = lyr.tile([128, nb], f32, name="mean", tag="mean")
            var = lyr.tile([128, nb], f32, name="var", tag="var")
            nc.vector.tensor_reduce(out=mean[:], in_=aggv[:, :, 0:fo],
                                    axis=mybir.AxisListType.X, op=Alu.add)
            for b in range(nb):
                sqb = lyr.tile([128, 128], f32, name="sqb", tag="sqb")
                ab = agg[:, b * 128:b * 128 + fo]
                tt(sqb[:, 0:fo], ab, ab, Alu.mult)
                nc.vector.tensor_reduce(out=var[:, b:b + 1], in_=sqb[:, 0:fo],
                                        axis=mybir.AxisListType.X, op=Alu.add)
            ts(mean[:], mean[:], 1.0 / fo, Alu.mult)
            ts(var[:], var[:], 1.0 / fo, Alu.mult)
            msq = lyr.tile([128, nb], f32, name="msq", tag="msq")
            tt(msq[:], mean[:], mean[:], Alu.mult)
            tt(var[:], var[:], msq[:], Alu.subtract)
            nc.scalar.activation(var[:], var[:], Act.Sqrt, bias=LN_EPS,
                                 scale=1.0)
            nc.vector.reciprocal(out=var[:], in_=var[:])
            for b in range(nb):
                ab = agg[:, b * 128:b * 128 + fo]
                ts(ab, ab, mean[:, b:b + 1], Alu.subtract)
                ts(ab, ab, var[:, b:b + 1], Alu.mult)
            dstT = h3T if l == 2 else other
            for b in range(nb):
                ptr = ppost.tile([fo, 128], f32, name="ptr", tag="pt")
                nc.tensor.transpose(ptr[0:fo, :],
                                    agg[:, b * 128:b * 128 + fo],
                                    idn[:, 0:128])
                nc.scalar.activation(
                    dstT[0:fo, b * 128:(b + 1) * 128], ptr[0:fo, :], Act.Gelu,
                    bias=W[f"nbeta{l}"][:, 0:1], scale=W[f"ng{l}"][:, 0:1])
            if l < 2:
                tt(other[:], other[:], cur[:], Alu.add)
                cur, other = other, cur
            else:
                for c0, cw in chunks:
                    pr = ps(OUT, name="p_res")
                    nc.tensor.matmul(pr[0:OUT, :cw], W["res_w"][:],
                                     cur[:, c0:c0 + cw], start=True,
                                     stop=True)
                    rr = lyr.tile([OUT, 512], f32, name="rr", tag="rr")
                    nc.scalar.activation(rr[:, :cw], pr[0:OUT, :cw],
                                         Act.Identity,
                                         bias=W["res_b"][:, 0:1], scale=1.0)
                    tt(h3T[:, c0:c0 + cw], h3T[:, c0:c0 + cw], rr[:, :cw],
                       Alu.add)
            _lyr_cm.__exit__(None, None, None)

        # ---- head: prototype attention, gate, classifier (chunk-wise)
        with tc.tile_pool(name="head", bufs=3) as wk:
            for c0, cw in chunks:
                pq = ps(OUT, name="p_q")
                nc.tensor.matmul(pq[0:OUT, :cw], W["q_w"][:],
                                 h3T[:, c0:c0 + cw], start=True, stop=True)
                qc = wk.tile([OUT, 512], f32, name="qc", tag="qc")
                nc.scalar.activation(qc[:, :cw], pq[0:OUT, :cw], Act.Identity,
                                     bias=W["q_b"][:, 0:1], scale=1.0)
                psc = ps(NPROTO, name="p_sc")
                nc.tensor.matmul(psc[0:NPROTO, :cw], W["proto_kT"][:],
                                 qc[:, :cw], start=True, stop=True)
                esc = wk.tile([NPROTO, 512], f32, name="esc", tag="esc")
                nc.scalar.activation(esc[:, :cw], psc[0:NPROTO, :cw],
                                     Act.Exp, bias=0.0, scale=1.0 / 8.0)
                pdn = ps(OUT, name="p_dn")
                nc.tensor.matmul(pdn[0:OUT, :cw], onesM[0:NPROTO, 0:OUT],
                                 esc[:, :cw], start=True, stop=True)
                rcc = wk.tile([OUT, 512], f32, name="rcc", tag="rcc")
                nc.vector.reciprocal(out=rcc[:, :cw], in_=pdn[0:OUT, :cw])
                fwc = wk.tile([1, 512], f32, name="fwc", tag="fwc")
                ts(fwc[:, :cw], pT[:, c0:c0 + cw], -0.5, Alu.add)
                fwd = wk.tile([1, 512], f32, name="fwd", tag="fwd")
                ts(fwd[:, :cw], fwc[:, :cw], -1.0, Alu.mult)
                tt(fwc[:, :cw], fwc[:, :cw], fwd[:, :cw], Alu.max)
                ts(fwc[:, :cw], fwc[:, :cw], -2.0, Alu.mult, 1.0, Alu.add)
                pfw = ps(OUT, name="p_fw")
                nc.tensor.matmul(pfw[0:OUT, :cw], onesM[0:1, 0:OUT],
                                 fwc[:, :cw], start=True, stop=True)
                pcx = ps(OUT, name="p_cx")
                nc.tensor.matmul(pcx[0:OUT, :cw], W["proto_v"][:],
                                 esc[:, :cw], start=True, stop=True)
                cxc = wk.tile([OUT, 512], f32, name="cxc", tag="cxc")
                tt(cxc[:, :cw], pcx[0:OUT, :cw], rcc[:, :cw], Alu.mult)
                tt(cxc[:, :cw], cxc[:, :cw], pfw[0:OUT, :cw], Alu.mult)
                h4c = wk.tile([OUT, 512], f32, name="h4c", tag="h4c")
                tt(h4c[:, :cw], h3T[:, c0:c0 + cw], cxc[:, :cw], Alu.add)
                pg1 = ps(16, name="p_g1")
                nc.tensor.matmul(pg1[0:16, :cw], W["gate_w1"][:],
                                 h4c[:, :cw], start=True, stop=False)
                nc.tensor.matmul(pg1[0:16, :cw], W["gate_w1p"][:],
                                 pT[:, c0:c0 + cw], start=False, stop=True)
                g1c = wk.tile([16, 512], f32, name="g1c", tag="g1c")
                nc.scalar.activation(g1c[:, :cw], pg1[0:16, :cw], Act.Relu,
                                     bias=W["gate_b1"][:, 0:1], scale=1.0)
                pg2 = ps(OUT, name="p_g2")
                nc.tensor.matmul(pg2[0:OUT, :cw], W["gate_w2"][:],
                                 g1c[:, :cw], start=True, stop=True)
                gvc = wk.tile([OUT, 512], f32, name="gvc", tag="gvc")
                nc.scalar.activation(gvc[:, :cw], pg2[0:OUT, :cw],
                                     Act.Sigmoid, bias=W["gate_b2"][:, 0:1],
                                     scale=1.0)
                tt(h4c[:, :cw], h4c[:, :cw], gvc[:, :cw], Alu.mult)
                pc1 = ps(32, name="p_c1")
                nc.tensor.matmul(pc1[0:32, :cw], W["cls_w1"][:],
                                 h4c[:, :cw], start=True, stop=True)
                c1c = wk.tile([32, 512], f32, name="c1c", tag="c1c")
                nc.scalar.activation(c1c[:, :cw], pc1[0:32, :cw], Act.Gelu,
                                     bias=W["cls_b1"][:, 0:1], scale=1.0)
                pc2 = ps(1, name="p_c2")
                nc.tensor.matmul(pc2[0:1, :cw], W["cls_w2"][:], c1c[:, :cw],
                                 start=True, stop=True)
                lgc = wk.tile([1, 512], f32, name="lgc", tag="lgc")
                nc.scalar.activation(lgc[:, :cw], pc2[0:1, :cw],
                                     Act.Identity, bias=W["cls_b2"][0:1, 0:1],
                                     scale=1.0)
                if c0 < cfg.npc:
                    w_out = min(cw, cfg.npc - c0)
                    nc.sync.dma_start(
                        out=out_ext[c0:c0 + w_out, :].rearrange(
                            "n one -> one n"),
                        in_=lgc[0:1, 0:w_out])
    nc.compile()
    return nc


# --------------------------------------------------------------------------
# entry point
# --------------------------------------------------------------------------

_CACHE = {}


def kernel(x, edge_index, params):
    cfg = FULL
    T, in_maps = prep_inputs(x, edge_index, params, cfg)
    key = ("prog", cfg.N, cfg.E, T)
    if key not in _CACHE:
        _CACHE[key] = build_program(cfg, T)
    nc = _CACHE[key]
    trace = bool(int(os.environ.get("DYGAT_TRACE", "0")))
    res = run_bass_kernel_spmd(nc, in_maps, list(range(NCORES)), trace=trace)
    if trace and res.exec_time_ns is not None:
        print(f"HW exec time: {res.exec_time_ns} ns")
    out = np.concatenate([res.results[c]["out"] for c in range(NCORES)],
                         axis=0)
    return out.astype(np.float32)
